# revision 19
# baseline (speedup 1.0000x reference)
"""Trainium2 Bass kernel for 2x2 sliding-window entropy (nn_Entropy).

ent[c,h',w'] = -sum_i p_i*log(p_i+eps),  p_i = w_i/(S+eps),  S = sum_i w_i
over the 4 elements of each 2x2 window of x (stride 1).

Identity (exact up to the inner +eps; ~1e-4 absolute):
    ent = u - (B + eps*u) * R
    u = ln(S+eps), R = exp(-u) = 1/(S+eps), B = box2x2(G), G = x*ln(x+eps),
    S = box2x2(x)

Per core x is (64,256,256) fp32 -> flat rows (c*h)=16384 x 256. g-blocks of
128 input rows stepping 127 (1-row overlap in the DMA access pattern)
produce 127 output rows each; 16383/127 = 129 blocks exactly.

Both box dimensions are computed on the PE: a [128,127] band matmul does the
vertical pair-sum, and a second matmul with the rhs shifted one column
accumulates into the same PSUM bank (start=False), yielding the full 2x2 box
directly in PSUM. Inputs stream as float32r (PE full rate; 11-bit mantissa,
end-to-end absmax ~6e-4 vs float32 reference). The remaining elementwise
work is ln/exp on ScalarE, (eps*u+B) and *R and u-t2 on DVE, G on GpSimd.

Sharding: pure data-parallel, batch dim (8) across the 8 cores.
"""
import numpy as np

B_FULL, C, H, W = 8, 64, 256, 256
HP, WP = H - 1, W - 1          # 255, 255
EPS = 1e-6
NCORES = 8

GROWS = 127                    # output rows per g-block
NG = (C * H - 1) // GROWS      # 16383/127 = 129 g-blocks
GPER = 8                       # g-blocks per super-block
PSUM_G = 4                     # g-blocks per PSUM tile (4 banks)
CPAD = 2 * W + 4               # per-g columns in comb (x | G | 4 pad)

_CACHE = {}


def _build(use_fp32r=True):
    import concourse.bacc as bacc
    import concourse.tile as tile
    import concourse.bass as bass
    from concourse import mybir

    f32 = mybir.dt.float32
    mmdt = mybir.dt.float32r if use_fp32r else f32
    nc = bacc.Bacc("TRN2", target_bir_lowering=False, debug=False)

    # input pre-transposed on host: x_t[p, k, w] = x[flat row 127k+p, w]
    # so one partition's g-blocks are contiguous -> 8KB DMA chunks
    x_d = nc.dram_tensor("x", [128 * NG * W], f32, kind="ExternalInput")
    band_d = nc.dram_tensor("band", [128, GROWS], f32, kind="ExternalInput")
    # raw block-dump output: row order (sb, p, g); host un-shuffles
    ent_d = nc.dram_tensor("ent", [NG * GROWS * WP], f32, kind="ExternalOutput")

    x_h = x_d[:].tensor
    ent_h = ent_d[:].tensor

    sblocks = [list(range(s, min(s + GPER, NG))) for s in range(0, NG, GPER)]

    with tile.TileContext(nc) as tc:
        with (
            tc.tile_pool(name="singles", bufs=1) as singles,
            tc.tile_pool(name="comb", bufs=2) as comb_p,
            tc.tile_pool(name="lt", bufs=2) as lt_p,
            tc.tile_pool(name="ps", bufs=2, space="PSUM") as ps_p,
            tc.tile_pool(name="sb8", bufs=2) as sb8_p,
            tc.tile_pool(name="entp", bufs=3) as ent_p,
        ):
            band = singles.tile([128, GROWS], mmdt)
            if use_fp32r:
                nc.gpsimd.dma_start(out=band, in_=band_d[:, :])
            else:
                nc.sync.dma_start(out=band, in_=band_d[:, :])
            eps_t = singles.tile([128, 1], f32)
            nc.vector.memset(eps_t, EPS)

            for gs in sblocks:
                gc = len(gs)
                k0 = gs[0]
                # unpadded per-partition-contiguous tiles (+4 pad cols at the
                # end for the shifted rhs of the last g-block)
                xt = comb_p.tile([128, GPER * W + 4], mmdt, tag="xt")
                Gt = comb_p.tile([128, GPER * W + 4], mmdt, tag="Gt")
                L = lt_p.tile([128, GPER * W], f32, tag="L")

                # load x rows 127k..127k+127 for each k (host pre-transposed:
                # x_t addr(p,k,w) = (p*NG + k)*W + w -> 8KB contiguous chunks)
                src = bass.AP(
                    tensor=x_h,
                    offset=k0 * W,
                    ap=[[NG * W, 128], [1, gc * W]],
                )
                # SWDGE cast fp32 -> fp32r during the DMA
                nc.gpsimd.dma_start(out=xt[:, 0:gc * W], in_=src)

                xs = xt[:, 0:gc * W].bitcast(f32)
                # L = ln(x + eps)   [ACT]
                nc.scalar.activation(
                    L[:, 0:gc * W], xs, mybir.ActivationFunctionType.Ln,
                    bias=eps_t,
                )
                # G = x * L (rounded to fp32r on write)   [GpSimd]
                nc.gpsimd.tensor_tensor(
                    Gt[:, 0:gc * W], xs, L[:, 0:gc * W], op=mybir.AluOpType.mult
                )

                u8 = sb8_p.tile([GROWS, gc, WP], f32, tag="u8")
                R8 = sb8_p.tile([GROWS, gc, WP], f32, tag="R8")
                t1 = sb8_p.tile([GROWS, gc, WP], f32, tag="t1")
                ent8 = ent_p.tile([GROWS, gc, WP], f32, tag="ent8")

                for c0 in range(0, gc, PSUM_G):
                    cc = min(PSUM_G, gc - c0)
                    ps = ps_p.tile([GROWS, cc, 2 * W], f32, tag="ps")
                    for j in range(cc):
                        g = c0 + j
                        # x box: vertical band, then horizontal shift accumulate
                        nc.tensor.matmul(
                            ps[:, j, 0:W], band, xt[:, g * W:(g + 1) * W],
                            start=True, stop=False,
                        )
                        nc.tensor.matmul(
                            ps[:, j, 0:W], band, xt[:, g * W + 1:(g + 1) * W + 1],
                            start=False, stop=False, skip_group_check=True,
                        )
                        # G box
                        nc.tensor.matmul(
                            ps[:, j, W:2 * W], band, Gt[:, g * W:(g + 1) * W],
                            start=True, stop=False, skip_group_check=True,
                        )
                        nc.tensor.matmul(
                            ps[:, j, W:2 * W], band, Gt[:, g * W + 1:(g + 1) * W + 1],
                            start=False, stop=True, skip_group_check=True,
                        )
                    # u = ln(S+eps)  (PSUM -> SBUF)   [ACT]
                    nc.scalar.activation(
                        u8[:, c0:c0 + cc, :], ps[:, :, 0:WP],
                        mybir.ActivationFunctionType.Ln, bias=eps_t[0:GROWS, :],
                    )
                    # t1 = eps*u + B  (B from PSUM)   [DVE]
                    nc.vector.scalar_tensor_tensor(
                        t1[:, c0:c0 + cc, :], u8[:, c0:c0 + cc, :], float(EPS),
                        ps[:, :, W:W + WP],
                        op0=mybir.AluOpType.mult, op1=mybir.AluOpType.add,
                    )

                # R = exp(-u)   [ACT]
                nc.scalar.activation(
                    R8, u8, mybir.ActivationFunctionType.Exp, scale=-1.0
                )
                # t2 = t1 * R   (in-place into t1)   [DVE]
                nc.vector.tensor_mul(t1, t1, R8)
                # ent = u - t2   [DVE]
                nc.vector.tensor_sub(ent8, u8, t1)

                # raw contiguous dump: row R = (k0/GPER)*127*GPER + p*gc + g,
                # 8KB contiguous per partition; host un-shuffles
                dst = bass.AP(
                    tensor=ent_h,
                    offset=k0 * GROWS * WP,
                    ap=[[gc * WP, GROWS], [1, gc * WP]],
                )
                nc.gpsimd.dma_start(
                    out=dst, in_=ent8.rearrange("p a b -> p (a b)")
                )

    nc.compile()
    return nc


def _band_np():
    a = np.zeros((128, GROWS), dtype=np.float32)
    for k in range(128):
        if k < GROWS:
            a[k, k] = 1.0
        if 0 < k <= GROWS:
            a[k, k - 1] = 1.0
    return a


def kernel(x: np.ndarray) -> np.ndarray:
    from concourse.bass_utils import run_bass_kernel_spmd

    assert x.shape == (B_FULL, C, H, W), x.shape
    if "nc" not in _CACHE:
        _CACHE["nc"] = _build()
    nc = _CACHE["nc"]

    band = _band_np()
    x = np.ascontiguousarray(x, dtype=np.float32)
    in_maps = []
    for i in range(NCORES):
        xf = x[i].reshape(C * H, W)
        rs = xf.strides[0]
        xt = np.lib.stride_tricks.as_strided(
            xf, shape=(128, NG, W), strides=(rs, GROWS * rs, xf.strides[1])
        )
        in_maps.append({
            "x": np.ascontiguousarray(xt).reshape(-1),
            "band": band,
        })
    res = run_bass_kernel_spmd(nc, in_maps, list(range(NCORES)))

    nfull = NG // GPER            # 16 full super-blocks
    out = np.empty((NCORES, C * H, WP), dtype=np.float32)
    for i in range(NCORES):
        raw = res.results[i]["ent"]
        a = raw[: nfull * GROWS * GPER * WP].reshape(nfull, GROWS, GPER, WP)
        a = a.transpose(0, 2, 1, 3).reshape(nfull * GPER * GROWS, WP)
        t = raw[nfull * GROWS * GPER * WP:].reshape(GROWS, WP)
        out[i, : NG * GROWS] = np.concatenate([a, t], axis=0)
    out = out.reshape(B_FULL, C, H, WP)[:, :, :HP, :]  # drop pad row 255
    return np.ascontiguousarray(out).reshape(B_FULL, C, HP * WP).astype(np.float32)
